# revision 2
# baseline (speedup 1.0000x reference)
"""LoRA-linear Trainium2 Bass kernel.

Computes, for T adapters: out[t] = x @ W.T + (x @ A_t.T) @ B_t.T + bias
  x: [B=4, S=4096, Din=1024]  -> tokens N = 16384
  W: [Dout=1024, Din=1024], bias: [Dout]
  lora_A: [NL=8, R=16, Din], lora_B: [NL=8, Dout, R], tuner_index: [T=4]
Output: [T, B, S, Dout] float32.

Sharding: data-parallel over tokens across 8 NeuronCores (2048 tokens/core);
W/bias/selected-LoRA replicated. Per core:
  lowT = (A_sel padded to 32-row groups).T-contracted with x -> [128, 2048]
  base = x @ W.T (+bias via a K=1 ones-row matmul) in PSUM, fp32r matmuls
  delta_t = lowT[32t:32t+16] @ B_t.T via K=16 row-group matmuls
  out_t = base + delta_t (VectorE), stored as contiguous 512KB blocks.
"""

import sys

if "/opt/trn_rl_repo" not in sys.path:
    sys.path.insert(0, "/opt/trn_rl_repo")

from contextlib import ExitStack

import numpy as np

import concourse.bacc as bacc
import concourse.bass as bass
import concourse.mybir as mybir
import concourse.tile as tile
from concourse import bass_utils

# Problem constants (hardcoded per spec).
B, S, DIN, DOUT, R, NL, T = 4, 4096, 1024, 1024, 16, 8, 4
NCORES = 8
NTOK = B * S                 # 16384
CTOK = NTOK // NCORES        # 2048 tokens per core
KT = DIN // 128              # 8 k-tiles
TOK_TILES = CTOK // 128      # 16
NCHUNK = 2                   # Dout chunks of 512
LOW_CHUNKS = CTOK // 512     # 4 token-chunks for the lowT matmuls

F32 = mybir.dt.float32
F32R = mybir.dt.float32r


def _build_program():
    nc = bacc.Bacc("TRN2", target_bir_lowering=False, debug=False,
                   num_devices=NCORES)

    xT = nc.dram_tensor("xT", [DIN, CTOK], F32R, kind="ExternalInput").ap()
    wt = nc.dram_tensor("wt", [DIN, DOUT], F32R, kind="ExternalInput").ap()
    atp = nc.dram_tensor("atp", [DIN, 128], F32R, kind="ExternalInput").ap()
    btp = nc.dram_tensor("btp", [128, DOUT], F32R, kind="ExternalInput").ap()
    biasr = nc.dram_tensor("biasr", [1, DOUT], F32R, kind="ExternalInput").ap()
    onesr = nc.dram_tensor("onesr", [1, 128], F32R, kind="ExternalInput").ap()
    out = nc.dram_tensor("out", [T, CTOK, DOUT], F32, kind="ExternalOutput").ap()

    with tile.TileContext(nc) as tc, ExitStack() as ctx:
        const = ctx.enter_context(tc.tile_pool(name="const", bufs=1))
        base_sb = ctx.enter_context(tc.tile_pool(name="base_sb", bufs=4))
        out_sb = ctx.enter_context(tc.tile_pool(name="out_sb", bufs=4))
        low_ps = ctx.enter_context(tc.tile_pool(name="low_ps", bufs=2, space="PSUM"))
        base_ps = ctx.enter_context(tc.tile_pool(name="base_ps", bufs=2, space="PSUM"))
        delta_ps = ctx.enter_context(tc.tile_pool(name="delta_ps", bufs=4, space="PSUM"))

        # Resident inputs.
        xt_t = []
        for k in range(KT):
            t_ = const.tile([128, CTOK], F32R, tag=f"xt{k}")
            nc.sync.dma_start(t_[:], xT[bass.ts(k, 128), :])
            xt_t.append(t_)
        wt_t = []
        for k in range(KT):
            t_ = const.tile([128, DOUT], F32R, tag=f"wt{k}")
            nc.sync.dma_start(t_[:], wt[bass.ts(k, 128), :])
            wt_t.append(t_)
        at_t = []
        for k in range(KT):
            t_ = const.tile([128, 128], F32R, tag=f"at{k}")
            nc.sync.dma_start(t_[:], atp[bass.ts(k, 128), :])
            at_t.append(t_)
        bt_s = const.tile([128, DOUT], F32R, tag="bt")
        nc.sync.dma_start(bt_s[:], btp[:, :])
        bias_s = const.tile([1, DOUT], F32R, tag="bias")
        nc.sync.dma_start(bias_s[:], biasr[:, :])
        ones_s = const.tile([1, 128], F32R, tag="ones")
        nc.sync.dma_start(ones_s[:], onesr[:, :])
        lowT_s = const.tile([128, CTOK], F32R, tag="lowT")

        # Phase 1: lowT[32t+j, tok] = sum_d A_sel[t, j, d] * x[tok, d].
        for c in range(LOW_CHUNKS):
            lp = low_ps.tile([128, 512], F32)
            for k in range(KT):
                nc.tensor.matmul(
                    lp[:],
                    at_t[k][:],
                    xt_t[k][:, bass.ts(c, 512)],
                    start=(k == 0), stop=(k == KT - 1),
                )
            nc.vector.tensor_copy(lowT_s[:, bass.ts(c, 512)], lp[:])

        # Phase 2: per token-tile, base then per-adapter outputs.
        for i in range(TOK_TILES):
            btiles = []
            for j in range(NCHUNK):
                bp = base_ps.tile([128, 512], F32)
                for k in range(KT):
                    nc.tensor.matmul(
                        bp[:],
                        xt_t[k][:, bass.ts(i, 128)],
                        wt_t[k][:, bass.ts(j, 512)],
                        start=(k == 0), stop=False,
                    )
                # bias: K=1 matmul of ones-column times the bias row.
                nc.tensor.matmul(
                    bp[:],
                    ones_s[:],
                    bias_s[:, bass.ts(j, 512)],
                    start=False, stop=True,
                )
                bsb = base_sb.tile([128, 512], F32)
                nc.scalar.copy(bsb[:], bp[:])
                btiles.append(bsb)

            for t in range(T):
                od = out_sb.tile([128, DOUT], F32)
                for j in range(NCHUNK):
                    dp = delta_ps.tile([128, 512], F32)
                    nc.tensor.matmul(
                        dp[:],
                        lowT_s[32 * t:32 * t + R, bass.ts(i, 128)],
                        bt_s[32 * t:32 * t + R, bass.ts(j, 512)],
                        start=True, stop=True,
                        tile_position=(32 * t, 0),
                    )
                    nc.vector.tensor_add(
                        od[:, bass.ts(j, 512)], btiles[j][:], dp[:]
                    )
                nc.sync.dma_start(out[t, bass.ts(i, 128), :], od[:])

    nc.compile()
    return nc


_NC = None


def _get_program():
    global _NC
    if _NC is None:
        _NC = _build_program()
    return _NC


def kernel(**inputs):
    x = np.ascontiguousarray(np.asarray(inputs["x"], dtype=np.float32))
    W = np.asarray(inputs["W"], dtype=np.float32)
    bias_v = np.asarray(inputs["bias"], dtype=np.float32)
    lora_A = np.asarray(inputs["lora_A"], dtype=np.float32)
    lora_B = np.asarray(inputs["lora_B"], dtype=np.float32)
    tuner_index = np.asarray(inputs["tuner_index"]).astype(np.int64)

    assert x.shape == (B, S, DIN) and W.shape == (DOUT, DIN)
    assert tuner_index.shape == (T,)

    A_sel = lora_A[tuner_index]          # [T, R, Din]
    B_sel = lora_B[tuner_index]          # [T, Dout, R]

    xT = np.ascontiguousarray(x.reshape(NTOK, DIN).T)      # [Din, Ntok]
    wt = np.ascontiguousarray(W.T)                          # [Din, Dout]
    atp = np.zeros((DIN, 128), np.float32)
    atp.reshape(DIN, T, 32)[:, :, :R] = A_sel.transpose(2, 0, 1)
    btp = np.zeros((128, DOUT), np.float32)
    btp.reshape(T, 32, DOUT)[:, :R, :] = B_sel.transpose(0, 2, 1)
    biasr = np.ascontiguousarray(bias_v.reshape(1, DOUT))
    onesr = np.ones((1, 128), np.float32)

    in_maps = []
    for c in range(NCORES):
        in_maps.append({
            "xT": np.ascontiguousarray(xT[:, c * CTOK:(c + 1) * CTOK]),
            "wt": wt,
            "atp": atp,
            "btp": btp,
            "biasr": biasr,
            "onesr": onesr,
        })

    nc = _get_program()
    res = bass_utils.run_bass_kernel_spmd(nc, in_maps, core_ids=list(range(NCORES)))

    out = np.empty((T, NTOK, DOUT), np.float32)
    for c in range(NCORES):
        out[:, c * CTOK:(c + 1) * CTOK, :] = res.results[c]["out"]
    return out.reshape(T, B, S, DOUT)


# revision 6
# speedup vs baseline: 1.1811x; 1.1811x over previous
"""LoRA-linear Trainium2 Bass kernel (v2: bf16 matmuls, dout-partition layout).

Computes, for T adapters: out[t] = x @ W.T + (x @ A_t.T) @ B_t.T + bias
  x: [B=4, S=4096, Din=1024]  -> tokens N = 16384
  W: [Dout=1024, Din=1024], bias: [Dout]
  lora_A: [NL=8, R=16, Din], lora_B: [NL=8, Dout, R], tuner_index: [T=4]
Output: [T, B, S, Dout] float32.

Sharding: data-parallel over tokens across 8 NeuronCores (2048 tokens/core);
W/bias/selected-LoRA replicated. Matmul inputs are cast to bf16 on the host
(halves load traffic, enables fast-weight-load); accumulation stays fp32.

Per-core layout puts Dout on PSUM partitions (out.T tiles [dout=128, tok]):
  lowT[32t+j, tok] = sum_d A_t[j,d] x[tok,d]   (PE, K=128 tiles)
  base.T[m] = W[m-tile] @ x.T  (PE, accumulate over 8 k-tiles; bias folded
              into the ScalarE PSUM->SBUF evacuation as a per-partition bias)
  delta.T[t,m] = B_t.T[:, m-tile].T-contract lowT_t  (K=16 row-group matmuls,
              tile_position=(32t,0), 4 adapters concurrent in the PE array)
  out.T[t,m] = base.T[m] + delta.T[t,m]  (VectorE tensor_tensor, PSUM+SBUF)
Stores are contiguous 1MB blocks of out.T; the host transposes back.
"""

import sys

if "/opt/trn_rl_repo" not in sys.path:
    sys.path.insert(0, "/opt/trn_rl_repo")

from contextlib import ExitStack

import ml_dtypes
import numpy as np

import concourse.bacc as bacc
import concourse.bass as bass
import concourse.mybir as mybir
import concourse.tile as tile
from concourse import bass_utils

# Problem constants (hardcoded per spec).
B, S, DIN, DOUT, R, NL, T = 4, 4096, 1024, 1024, 16, 8, 4
NCORES = 8
NTOK = B * S                 # 16384
CTOK = NTOK // NCORES        # 2048 tokens per core
KT = DIN // 128              # 8 k-tiles
MT = DOUT // 128             # 8 dout-tiles
NC_CHUNK = CTOK // 512       # 4 token-chunks of 512

F32 = mybir.dt.float32
BF16 = mybir.dt.bfloat16
NPBF16 = ml_dtypes.bfloat16


def _build_program():
    nc = bacc.Bacc("TRN2", target_bir_lowering=False, debug=False,
                   num_devices=NCORES)

    xt = nc.dram_tensor("xt", [DIN, CTOK], BF16, kind="ExternalInput").ap()
    wt = nc.dram_tensor("wt", [DIN, DOUT], BF16, kind="ExternalInput").ap()
    atp = nc.dram_tensor("atp", [DIN, 128], BF16, kind="ExternalInput").ap()
    btp = nc.dram_tensor("btp", [128, DOUT], BF16, kind="ExternalInput").ap()
    biasc = nc.dram_tensor("biasc", [128, MT], F32, kind="ExternalInput").ap()
    out = nc.dram_tensor("out", [T, MT, 128, CTOK], F32,
                         kind="ExternalOutput").ap()

    with tile.TileContext(nc) as tc, ExitStack() as ctx:
        const = ctx.enter_context(tc.tile_pool(name="const", bufs=1))
        base_sb = ctx.enter_context(tc.tile_pool(name="base_sb", bufs=3))
        out_sb = ctx.enter_context(tc.tile_pool(name="out_sb", bufs=2))
        low_ps = ctx.enter_context(tc.tile_pool(name="low_ps", bufs=2, space="PSUM"))
        base_ps = ctx.enter_context(tc.tile_pool(name="base_ps", bufs=3, space="PSUM"))
        delta_ps = ctx.enter_context(tc.tile_pool(name="delta_ps", bufs=3, space="PSUM"))

        # Small constants first so compute can start early.
        at_t = []
        for k in range(KT):
            t_ = const.tile([128, 128], BF16, tag=f"at{k}")
            nc.sync.dma_start(t_[:], atp[bass.ts(k, 128), :])
            at_t.append(t_)
        bt_s = const.tile([128, DOUT], BF16, tag="bt")
        nc.sync.dma_start(bt_s[:], btp[:, :])
        bias_s = const.tile([128, MT], F32, tag="bias")
        nc.sync.dma_start(bias_s[:], biasc[:, :])
        xt_t = []
        wt_t = []
        for k in range(KT):
            tx = const.tile([128, CTOK], BF16, tag=f"xt{k}")
            nc.sync.dma_start(tx[:], xt[bass.ts(k, 128), :])
            xt_t.append(tx)
            tw = const.tile([128, DOUT], BF16, tag=f"wt{k}")
            nc.sync.dma_start(tw[:], wt[bass.ts(k, 128), :])
            wt_t.append(tw)
        lowT_s = const.tile([128, CTOK], BF16, tag="lowT")

        # Phase 1: lowT[32t+j, tok] = sum_d A_sel[t, j, d] * x[tok, d].
        for c in range(NC_CHUNK):
            lp = low_ps.tile([128, 512], F32)
            for k in range(KT):
                nc.tensor.matmul(
                    lp[:],
                    at_t[k][:],
                    xt_t[k][:, bass.ts(c, 512)],
                    start=(k == 0), stop=(k == KT - 1),
                )
            nc.vector.tensor_copy(lowT_s[:, bass.ts(c, 512)], lp[:])

        # Phase 2: per dout-tile m: base, then per-adapter outputs.
        for m in range(MT):
            bsb = base_sb.tile([128, CTOK], F32)
            for c in range(NC_CHUNK):
                bp = base_ps.tile([128, 512], F32)
                for k in range(KT):
                    nc.tensor.matmul(
                        bp[:],
                        wt_t[k][:, bass.ts(m, 128)],
                        xt_t[k][:, bass.ts(c, 512)],
                        start=(k == 0), stop=(k == KT - 1),
                    )
                # Evacuate with the per-partition bias folded in.
                nc.scalar.activation(
                    bsb[:, bass.ts(c, 512)], bp[:],
                    mybir.ActivationFunctionType.Identity,
                    bias=bias_s[:, m:m + 1],
                )

            ods = [out_sb.tile([128, CTOK], F32, tag=f"od{t}", name=f"od{t}_{m}")
                   for t in range(T)]
            for c in range(NC_CHUNK):
                for t in range(T):
                    dp = delta_ps.tile([128, 512], F32)
                    nc.tensor.matmul(
                        dp[:],
                        bt_s[32 * t:32 * t + R, bass.ts(m, 128)],
                        lowT_s[32 * t:32 * t + R, bass.ts(c, 512)],
                        start=True, stop=True,
                        tile_position=(32 * t, 0),
                    )
                    nc.vector.tensor_add(
                        ods[t][:, bass.ts(c, 512)],
                        bsb[:, bass.ts(c, 512)], dp[:],
                    )
            for t in range(T):
                nc.sync.dma_start(out[t, m, :, :], ods[t][:])

    nc.compile()
    return nc


_NC = None


def _get_program():
    global _NC
    if _NC is None:
        _NC = _build_program()
    return _NC


def kernel(**inputs):
    x = np.ascontiguousarray(np.asarray(inputs["x"], dtype=np.float32))
    W = np.asarray(inputs["W"], dtype=np.float32)
    bias_v = np.asarray(inputs["bias"], dtype=np.float32)
    lora_A = np.asarray(inputs["lora_A"], dtype=np.float32)
    lora_B = np.asarray(inputs["lora_B"], dtype=np.float32)
    tuner_index = np.asarray(inputs["tuner_index"]).astype(np.int64)

    assert x.shape == (B, S, DIN) and W.shape == (DOUT, DIN)
    assert tuner_index.shape == (T,)

    A_sel = lora_A[tuner_index]          # [T, R, Din]
    B_sel = lora_B[tuner_index]          # [T, Dout, R]

    xT = np.ascontiguousarray(x.reshape(NTOK, DIN).T).astype(NPBF16)
    wt = np.ascontiguousarray(W.T).astype(NPBF16)       # [Din, Dout]
    atp = np.zeros((DIN, 128), NPBF16)
    atp.reshape(DIN, T, 32)[:, :, :R] = A_sel.transpose(2, 0, 1).astype(NPBF16)
    btp = np.zeros((128, DOUT), NPBF16)
    btp.reshape(T, 32, DOUT)[:, :R, :] = B_sel.transpose(0, 2, 1).astype(NPBF16)
    biasc = np.ascontiguousarray(bias_v.reshape(MT, 128).T)   # [128, MT]

    in_maps = []
    for c in range(NCORES):
        in_maps.append({
            "xt": np.ascontiguousarray(xT[:, c * CTOK:(c + 1) * CTOK]),
            "wt": wt,
            "atp": atp,
            "btp": btp,
            "biasc": biasc,
        })

    nc = _get_program()
    res = bass_utils.run_bass_kernel_spmd(nc, in_maps, core_ids=list(range(NCORES)))

    big = np.empty((T, MT, 128, NTOK), np.float32)
    for c in range(NCORES):
        big[:, :, :, c * CTOK:(c + 1) * CTOK] = res.results[c]["out"]
    # [T, m, p, tok] -> [T, tok, m*128+p]
    full = np.ascontiguousarray(big.transpose(0, 3, 1, 2))
    return full.reshape(T, B, S, DOUT)


# revision 8
# speedup vs baseline: 1.1931x; 1.0102x over previous
"""LoRA-linear Trainium2 Bass kernel (v3: bf16, pipelined, shared PSUM pool).

Computes, for T adapters: out[t] = x @ W.T + (x @ A_t.T) @ B_t.T + bias
Output: [T, B, S, Dout] float32.

Sharding: data-parallel over tokens across 8 NeuronCores (2048 tokens/core);
W/bias/selected-LoRA replicated. Matmul inputs are cast to bf16 on the host
(halves load traffic, enables fast-weight-load); accumulation stays fp32.

Per-core layout puts Dout on PSUM partitions (out.T tiles [dout=128, tok]):
  lowT[32t+j, tok] = sum_d A_t[j,d] x[tok,d]   (PE, k-major so compute starts
              on the first arriving x k-tile)
  base.T[m] = W[m-tile] @ x.T  (PE, accumulate over 8 k-tiles; bias folded
              into the ScalarE PSUM->SBUF evacuation as a per-partition bias)
  delta.T[t,m] = B_t.T[:, m-tile].T-contract lowT_t  (K=16 row-group matmuls)
  out.T[t,m] = base.T[m] + delta.T[t,m]  (VectorE tensor_tensor, PSUM+SBUF)
Delta matmuls for dout-tile m-1 are emitted after the base matmuls of tile m
(software pipelining) so the PE never stalls waiting for VectorE adds and the
HAM clock gate stays warm. Stores are contiguous 1MB blocks of out.T; the
host transposes back.
"""

import sys

if "/opt/trn_rl_repo" not in sys.path:
    sys.path.insert(0, "/opt/trn_rl_repo")

from contextlib import ExitStack

import ml_dtypes
import numpy as np

import concourse.bacc as bacc
import concourse.bass as bass
import concourse.mybir as mybir
import concourse.tile as tile
from concourse import bass_utils

# Problem constants (hardcoded per spec).
B, S, DIN, DOUT, R, NL, T = 4, 4096, 1024, 1024, 16, 8, 4
NCORES = 8
NTOK = B * S                 # 16384
CTOK = NTOK // NCORES        # 2048 tokens per core
KT = DIN // 128              # 8 k-tiles
MT = DOUT // 128             # 8 dout-tiles
NC_CHUNK = CTOK // 512       # 4 token-chunks of 512

F32 = mybir.dt.float32
BF16 = mybir.dt.bfloat16
NPBF16 = ml_dtypes.bfloat16


def _build_program():
    nc = bacc.Bacc("TRN2", target_bir_lowering=False, debug=False,
                   num_devices=NCORES)

    xt = nc.dram_tensor("xt", [DIN, CTOK], BF16, kind="ExternalInput").ap()
    wt = nc.dram_tensor("wt", [DIN, DOUT], BF16, kind="ExternalInput").ap()
    atp = nc.dram_tensor("atp", [DIN, 128], BF16, kind="ExternalInput").ap()
    btp = nc.dram_tensor("btp", [128, DOUT], BF16, kind="ExternalInput").ap()
    biasc = nc.dram_tensor("biasc", [128, MT], F32, kind="ExternalInput").ap()
    out = nc.dram_tensor("out", [T, MT, 128, CTOK], F32,
                         kind="ExternalOutput").ap()

    with tile.TileContext(nc) as tc, ExitStack() as ctx:
        const = ctx.enter_context(tc.tile_pool(name="const", bufs=1))
        base_sb = ctx.enter_context(tc.tile_pool(name="base_sb", bufs=3))
        out_sb = ctx.enter_context(tc.tile_pool(name="out_sb", bufs=2))
        ps = ctx.enter_context(tc.tile_pool(name="ps", bufs=8, space="PSUM"))

        # Small constants first so compute can start early.
        at_t = []
        for k in range(KT):
            t_ = const.tile([128, 128], BF16, tag=f"at{k}")
            nc.sync.dma_start(t_[:], atp[bass.ts(k, 128), :])
            at_t.append(t_)
        bt_s = const.tile([128, DOUT], BF16, tag="bt")
        nc.sync.dma_start(bt_s[:], btp[:, :])
        bias_s = const.tile([128, MT], F32, tag="bias")
        nc.sync.dma_start(bias_s[:], biasc[:, :])
        xt_t = []
        wt_t = []
        for k in range(KT):
            tx = const.tile([128, CTOK], BF16, tag=f"xt{k}")
            nc.sync.dma_start(tx[:], xt[bass.ts(k, 128), :])
            xt_t.append(tx)
            tw = const.tile([128, DOUT], BF16, tag=f"wt{k}")
            nc.sync.dma_start(tw[:], wt[bass.ts(k, 128), :])
            wt_t.append(tw)
        lowT_s = const.tile([128, CTOK], BF16, tag="lowT")

        # Phase 1 (k-major): lowT[32t+j, tok] = sum_d A_sel[t,j,d] x[tok,d].
        lps = [ps.tile([128, 512], F32, tag="ps", name=f"lp{c}")
               for c in range(NC_CHUNK)]
        for k in range(KT):
            for c in range(NC_CHUNK):
                nc.tensor.matmul(
                    lps[c][:],
                    at_t[k][:],
                    xt_t[k][:, bass.ts(c, 512)],
                    start=(k == 0), stop=(k == KT - 1),
                )
        for c in range(NC_CHUNK):
            nc.scalar.copy(lowT_s[:, bass.ts(c, 512)], lps[c][:])

        # Phase 2, software-pipelined: base(m) emitted before delta(m-1).
        def emit_base(m):
            bsb = base_sb.tile([128, CTOK], F32, tag="bsb", name=f"bsb{m}")
            for c in range(NC_CHUNK):
                bp = ps.tile([128, 512], F32, tag="ps", name=f"bp{m}_{c}")
                for k in range(KT):
                    nc.tensor.matmul(
                        bp[:],
                        wt_t[k][:, bass.ts(m, 128)],
                        xt_t[k][:, bass.ts(c, 512)],
                        start=(k == 0), stop=(k == KT - 1),
                    )
                # Evacuate with the per-partition bias folded in.
                nc.scalar.activation(
                    bsb[:, bass.ts(c, 512)], bp[:],
                    mybir.ActivationFunctionType.Identity,
                    bias=bias_s[:, m:m + 1],
                )
            return bsb

        def emit_delta(m, bsb):
            ods = [out_sb.tile([128, CTOK], F32, tag=f"od{t}", name=f"od{t}_{m}")
                   for t in range(T)]
            for c in range(NC_CHUNK):
                for t in range(T):
                    dp = ps.tile([128, 512], F32, tag="ps", name=f"dp{m}_{c}_{t}")
                    nc.tensor.matmul(
                        dp[:],
                        bt_s[32 * t:32 * t + R, bass.ts(m, 128)],
                        lowT_s[32 * t:32 * t + R, bass.ts(c, 512)],
                        start=True, stop=True,
                        tile_position=(32 * t, 0),
                    )
                    nc.vector.tensor_add(
                        ods[t][:, bass.ts(c, 512)],
                        bsb[:, bass.ts(c, 512)], dp[:],
                    )
            for t in range(T):
                nc.sync.dma_start(out[t, m, :, :], ods[t][:])

        prev = None
        for m in range(MT):
            bsb = emit_base(m)
            if prev is not None:
                emit_delta(m - 1, prev)
            prev = bsb
        emit_delta(MT - 1, prev)

    nc.compile()
    return nc


_NC = None


def _get_program():
    global _NC
    if _NC is None:
        _NC = _build_program()
    return _NC


def kernel(**inputs):
    x = np.ascontiguousarray(np.asarray(inputs["x"], dtype=np.float32))
    W = np.asarray(inputs["W"], dtype=np.float32)
    bias_v = np.asarray(inputs["bias"], dtype=np.float32)
    lora_A = np.asarray(inputs["lora_A"], dtype=np.float32)
    lora_B = np.asarray(inputs["lora_B"], dtype=np.float32)
    tuner_index = np.asarray(inputs["tuner_index"]).astype(np.int64)

    assert x.shape == (B, S, DIN) and W.shape == (DOUT, DIN)
    assert tuner_index.shape == (T,)

    A_sel = lora_A[tuner_index]          # [T, R, Din]
    B_sel = lora_B[tuner_index]          # [T, Dout, R]

    xT = np.ascontiguousarray(x.reshape(NTOK, DIN).T).astype(NPBF16)
    wt = np.ascontiguousarray(W.T).astype(NPBF16)       # [Din, Dout]
    atp = np.zeros((DIN, 128), NPBF16)
    atp.reshape(DIN, T, 32)[:, :, :R] = A_sel.transpose(2, 0, 1).astype(NPBF16)
    btp = np.zeros((128, DOUT), NPBF16)
    btp.reshape(T, 32, DOUT)[:, :R, :] = B_sel.transpose(0, 2, 1).astype(NPBF16)
    biasc = np.ascontiguousarray(bias_v.reshape(MT, 128).T)   # [128, MT]

    in_maps = []
    for c in range(NCORES):
        in_maps.append({
            "xt": np.ascontiguousarray(xT[:, c * CTOK:(c + 1) * CTOK]),
            "wt": wt,
            "atp": atp,
            "btp": btp,
            "biasc": biasc,
        })

    nc = _get_program()
    res = bass_utils.run_bass_kernel_spmd(nc, in_maps, core_ids=list(range(NCORES)))

    big = np.empty((T, MT, 128, NTOK), np.float32)
    for c in range(NCORES):
        big[:, :, :, c * CTOK:(c + 1) * CTOK] = res.results[c]["out"]
    # [T, m, p, tok] -> [T, tok, m*128+p]
    full = np.ascontiguousarray(big.transpose(0, 3, 1, 2))
    return full.reshape(T, B, S, DOUT)


# revision 9
# speedup vs baseline: 1.4429x; 1.2093x over previous
"""LoRA-linear Trainium2 Bass kernel (v3: bf16, pipelined, shared PSUM pool).

Computes, for T adapters: out[t] = x @ W.T + (x @ A_t.T) @ B_t.T + bias
Output: [T, B, S, Dout] float32.

Sharding: data-parallel over tokens across 8 NeuronCores (2048 tokens/core);
W/bias/selected-LoRA replicated. Matmul inputs are cast to bf16 on the host
(halves load traffic, enables fast-weight-load); accumulation stays fp32.

Per-core layout puts Dout on PSUM partitions (out.T tiles [dout=128, tok]):
  lowT[32t+j, tok] = sum_d A_t[j,d] x[tok,d]   (PE, k-major so compute starts
              on the first arriving x k-tile)
  base.T[m] = W[m-tile] @ x.T  (PE, accumulate over 8 k-tiles; bias folded
              into the ScalarE PSUM->SBUF evacuation as a per-partition bias)
  delta.T[t,m] = B_t.T[:, m-tile].T-contract lowT_t  (K=16 row-group matmuls)
  out.T[t,m] = base.T[m] + delta.T[t,m]  (VectorE tensor_tensor, PSUM+SBUF)
Delta matmuls for dout-tile m-1 are emitted after the base matmuls of tile m
(software pipelining) so the PE never stalls waiting for VectorE adds and the
HAM clock gate stays warm. Stores are contiguous 1MB blocks of out.T; the
host transposes back.
"""

import sys

if "/opt/trn_rl_repo" not in sys.path:
    sys.path.insert(0, "/opt/trn_rl_repo")

from contextlib import ExitStack

import ml_dtypes
import numpy as np

import concourse.bacc as bacc
import concourse.bass as bass
import concourse.mybir as mybir
import concourse.tile as tile
from concourse import bass_utils

# Problem constants (hardcoded per spec).
B, S, DIN, DOUT, R, NL, T = 4, 4096, 1024, 1024, 16, 8, 4
NCORES = 8
NTOK = B * S                 # 16384
CTOK = NTOK // NCORES        # 2048 tokens per core
KT = DIN // 128              # 8 k-tiles
MT = DOUT // 128             # 8 dout-tiles
NC_CHUNK = CTOK // 512       # 4 token-chunks of 512

F32 = mybir.dt.float32
BF16 = mybir.dt.bfloat16
NPBF16 = ml_dtypes.bfloat16


def _build_program():
    nc = bacc.Bacc("TRN2", target_bir_lowering=False, debug=False,
                   num_devices=NCORES)

    xt = nc.dram_tensor("xt", [DIN, CTOK], BF16, kind="ExternalInput").ap()
    wt = nc.dram_tensor("wt", [DIN, DOUT], BF16, kind="ExternalInput").ap()
    atp = nc.dram_tensor("atp", [DIN, 128], BF16, kind="ExternalInput").ap()
    btp = nc.dram_tensor("btp", [128, DOUT], BF16, kind="ExternalInput").ap()
    biasc = nc.dram_tensor("biasc", [128, MT], F32, kind="ExternalInput").ap()
    out = nc.dram_tensor("out", [T, MT, 128, CTOK], F32,
                         kind="ExternalOutput").ap()

    with tile.TileContext(nc) as tc, ExitStack() as ctx:
        const = ctx.enter_context(tc.tile_pool(name="const", bufs=1))
        base_sb = ctx.enter_context(tc.tile_pool(name="base_sb", bufs=3))
        out_sb = ctx.enter_context(tc.tile_pool(name="out_sb", bufs=2))
        bp_ps = ctx.enter_context(tc.tile_pool(name="bp_ps", bufs=2, space="PSUM"))
        dp_ps = ctx.enter_context(tc.tile_pool(name="dp_ps", bufs=6, space="PSUM"))

        # Small constants first so compute can start early.
        at_t = []
        for k in range(KT):
            t_ = const.tile([128, 128], BF16, tag=f"at{k}")
            nc.sync.dma_start(t_[:], atp[bass.ts(k, 128), :])
            at_t.append(t_)
        bt_s = const.tile([128, DOUT], BF16, tag="bt")
        nc.sync.dma_start(bt_s[:], btp[:, :])
        bias_s = const.tile([128, MT], F32, tag="bias")
        nc.sync.dma_start(bias_s[:], biasc[:, :])
        xt_t = []
        wt_t = []
        for k in range(KT):
            tx = const.tile([128, CTOK], BF16, tag=f"xt{k}")
            nc.sync.dma_start(tx[:], xt[bass.ts(k, 128), :])
            xt_t.append(tx)
            tw = const.tile([128, DOUT], BF16, tag=f"wt{k}")
            nc.sync.dma_start(tw[:], wt[bass.ts(k, 128), :])
            wt_t.append(tw)
        lowT_s = const.tile([128, CTOK], BF16, tag="lowT")

        # Phase 1 (k-major): lowT[32t+j, tok] = sum_d A_sel[t,j,d] x[tok,d].
        lps = [dp_ps.tile([128, 512], F32, tag="dp", name=f"lp{c}")
               for c in range(NC_CHUNK)]
        for k in range(KT):
            for c in range(NC_CHUNK):
                nc.tensor.matmul(
                    lps[c][:],
                    at_t[k][:],
                    xt_t[k][:, bass.ts(c, 512)],
                    start=(k == 0), stop=(k == KT - 1),
                )
        for c in range(NC_CHUNK):
            nc.scalar.copy(lowT_s[:, bass.ts(c, 512)], lps[c][:])

        # Phase 2, software-pipelined chunk-wise: base(m) chunk c is emitted
        # before delta(m-1) chunk c so the PE always has a dense base group to
        # chew while VectorE drains the previous delta bank.
        def emit_base_chunk(m, bsb, c):
            bp = bp_ps.tile([128, 512], F32, tag="bp", name=f"bp{m}_{c}")
            for k in range(KT):
                nc.tensor.matmul(
                    bp[:],
                    wt_t[k][:, bass.ts(m, 128)],
                    xt_t[k][:, bass.ts(c, 512)],
                    start=(k == 0), stop=(k == KT - 1),
                )
            # Evacuate with the per-partition bias folded in.
            nc.scalar.activation(
                bsb[:, bass.ts(c, 512)], bp[:],
                mybir.ActivationFunctionType.Identity,
                bias=bias_s[:, m:m + 1],
            )

        def emit_delta_chunk(m, bsb, ods, c):
            for t in range(T):
                dp = dp_ps.tile([128, 512], F32, tag="dp", name=f"dp{m}_{c}_{t}")
                nc.tensor.matmul(
                    dp[:],
                    bt_s[32 * t:32 * t + R, bass.ts(m, 128)],
                    lowT_s[32 * t:32 * t + R, bass.ts(c, 512)],
                    start=True, stop=True,
                    tile_position=(32 * t, 0),
                )
                nc.vector.tensor_add(
                    ods[t][:, bass.ts(c, 512)],
                    bsb[:, bass.ts(c, 512)], dp[:],
                )

        def make_ods(m):
            return [out_sb.tile([128, CTOK], F32, tag=f"od{t}", name=f"od{t}_{m}")
                    for t in range(T)]

        def store_ods(m, ods):
            for t in range(T):
                nc.sync.dma_start(out[t, m, :, :], ods[t][:])

        prev_bsb = None
        prev_ods = None
        for m in range(MT):
            bsb = base_sb.tile([128, CTOK], F32, tag="bsb", name=f"bsb{m}")
            for c in range(NC_CHUNK):
                emit_base_chunk(m, bsb, c)
                if prev_bsb is not None:
                    emit_delta_chunk(m - 1, prev_bsb, prev_ods, c)
            if prev_ods is not None:
                store_ods(m - 1, prev_ods)
            prev_bsb, prev_ods = bsb, make_ods(m)
        for c in range(NC_CHUNK):
            emit_delta_chunk(MT - 1, prev_bsb, prev_ods, c)
        store_ods(MT - 1, prev_ods)

    nc.compile()
    return nc


_NC = None


def _get_program():
    global _NC
    if _NC is None:
        _NC = _build_program()
    return _NC


def kernel(**inputs):
    x = np.ascontiguousarray(np.asarray(inputs["x"], dtype=np.float32))
    W = np.asarray(inputs["W"], dtype=np.float32)
    bias_v = np.asarray(inputs["bias"], dtype=np.float32)
    lora_A = np.asarray(inputs["lora_A"], dtype=np.float32)
    lora_B = np.asarray(inputs["lora_B"], dtype=np.float32)
    tuner_index = np.asarray(inputs["tuner_index"]).astype(np.int64)

    assert x.shape == (B, S, DIN) and W.shape == (DOUT, DIN)
    assert tuner_index.shape == (T,)

    A_sel = lora_A[tuner_index]          # [T, R, Din]
    B_sel = lora_B[tuner_index]          # [T, Dout, R]

    xT = np.ascontiguousarray(x.reshape(NTOK, DIN).T).astype(NPBF16)
    wt = np.ascontiguousarray(W.T).astype(NPBF16)       # [Din, Dout]
    atp = np.zeros((DIN, 128), NPBF16)
    atp.reshape(DIN, T, 32)[:, :, :R] = A_sel.transpose(2, 0, 1).astype(NPBF16)
    btp = np.zeros((128, DOUT), NPBF16)
    btp.reshape(T, 32, DOUT)[:, :R, :] = B_sel.transpose(0, 2, 1).astype(NPBF16)
    biasc = np.ascontiguousarray(bias_v.reshape(MT, 128).T)   # [128, MT]

    in_maps = []
    for c in range(NCORES):
        in_maps.append({
            "xt": np.ascontiguousarray(xT[:, c * CTOK:(c + 1) * CTOK]),
            "wt": wt,
            "atp": atp,
            "btp": btp,
            "biasc": biasc,
        })

    nc = _get_program()
    res = bass_utils.run_bass_kernel_spmd(nc, in_maps, core_ids=list(range(NCORES)))

    big = np.empty((T, MT, 128, NTOK), np.float32)
    for c in range(NCORES):
        big[:, :, :, c * CTOK:(c + 1) * CTOK] = res.results[c]["out"]
    # [T, m, p, tok] -> [T, tok, m*128+p]
    full = np.ascontiguousarray(big.transpose(0, 3, 1, 2))
    return full.reshape(T, B, S, DOUT)
